# revision 14
# baseline (speedup 1.0000x reference)
"""Sparse attention (topk=64) Trainium2 kernel, 8-core SPMD.

qkv = x @ w_qkv.T with a RAW reshape to (3,B,H,N,hd): each (s,b2,h) slice is
a CONTIGUOUS 32768-float chunk of the flat qkv buffer. Core j owns
pseudo-batch b2=j (12 heads) -> communication-free across cores.

SPMD phase trick: chunk offsets within a per-core x row-slice have sub-row
phase 384*((2g+j)%3) elems (g=0,1,2 for q,k,v). Host places group g into
slab s=(2g+j)%3 so slab s always has phase 384*s in the shared graph;
per-core 0/1 masks select which slab plays the q/k/v role.

Per head: scores f32 via PE (q,k PE-transposed from row tiles), diagonal
masked by gpsimd.affine_select, top-64 via 8 rounds of DVE max8 (+7
match_replace), softmax = ACT exp(scale*rq*(s-max)) masked by s>=kth fused
in one scalar_tensor_tensor with rowsum accumulation; attn rows normalized,
cast bf16, bounced via DRAM + xbar transpose; attn.T@v and final proj on PE.
q's l2-norm folds into the exp scale (row scaling preserves the topk set);
k is l2-normalized in row layout before transposition.
"""

import sys
import numpy as np

sys.path.insert(0, "/opt/trn_rl_repo")

HEAD_DIM = 32
NUM_HEADS = 12
TOPK = 64
NUM_MEM = 16
B, Hh, Ww, D = 8, 32, 32, 384
N = Hh * Ww  # 1024
INNER = NUM_HEADS * HEAD_DIM  # 384
NQKV = 3 * INNER  # 1152
NEG = float(-np.finfo(np.float32).max)
CHUNK = N * HEAD_DIM  # 32768 floats per (s,b2,h) chunk
SLAB_ROWS = 344
XPAD = 1152
M_FULL = N + NUM_MEM  # 1040
M_PAD = 1152

STATS = {}


def _build_graph(scale_vals):
    import concourse.bass as bass
    import concourse.bacc as bacc
    import concourse.mybir as mybir
    from concourse.tile import TileContext

    fp32 = mybir.dt.float32
    bf16 = mybir.dt.bfloat16
    AF = mybir.ActivationFunctionType
    AO = mybir.AluOpType

    nc = bacc.Bacc()

    xt = nc.declare_dram_parameter("xt", [D, XPAD], fp32, isOutput=False)
    wq = nc.declare_dram_parameter("wq", [D, NQKV], fp32, isOutput=False)
    wp = nc.declare_dram_parameter("wp", [INNER, D], fp32, isOutput=False)
    mrow = nc.declare_dram_parameter("mrow", [128, 9], fp32, isOutput=False)
    ident = nc.declare_dram_parameter("ident", [128, 128], fp32, isOutput=False)
    memknt = nc.declare_dram_parameter("memknt", [NUM_HEADS * 32, NUM_MEM], fp32,
                                       isOutput=False)
    memv = nc.declare_dram_parameter("memv", [NUM_HEADS * NUM_MEM, 32], bf16,
                                     isOutput=False)
    out_ext = nc.declare_dram_parameter("out", [D, N], fp32, isOutput=True)

    with TileContext(nc) as tc:
        with (
            tc.tile_pool(name="wts", bufs=1) as wts,
            tc.tile_pool(name="st1", bufs=2) as st1,
            tc.tile_pool(name="sc", bufs=2) as scp,
            tc.tile_pool(name="small", bufs=2) as sm,
            tc.tile_pool(name="att", bufs=2) as attp,
            tc.tile_pool(name="dram", bufs=1, space="DRAM") as dr1,
            tc.tile_pool(name="dram2", bufs=2, space="DRAM") as dr2,
            tc.tile_pool(name="big_ps", bufs=2, space="PSUM") as bps,
            tc.tile_pool(name="sm_ps", bufs=2, space="PSUM") as sps,
        ):
            ydram = dr1.tile([XPAD * NQKV], fp32, tag="ydram")

            # ---------------- stage 1: QKV matmul -> ydram ------------------
            xt_sb = wts.tile([128, 3 * XPAD], fp32, tag="xt")
            wq_sb = wts.tile([128, 3 * NQKV], fp32, tag="wq")
            nc.sync.dma_start(xt_sb[:].rearrange("p (k c) -> p k c", c=XPAD),
                              xt[:].rearrange("(k p) c -> p k c", p=128))
            nc.sync.dma_start(wq_sb[:].rearrange("p (k c) -> p k c", c=NQKV),
                              wq[:].rearrange("(k p) c -> p k c", p=128))
            yv = ydram[:].rearrange("(r c) -> r c", c=NQKV)
            for m in range(9):
                y_sb = st1.tile([128, NQKV], fp32, tag="y")
                for (n0, nw) in ((0, 512), (512, 512), (1024, 128)):
                    ps = sps.tile([128, 512], fp32, tag="ps_small")
                    for k in range(3):
                        nc.tensor.matmul(
                            ps[:, :nw],
                            xt_sb[:, k * XPAD + m * 128: k * XPAD + (m + 1) * 128],
                            wq_sb[:, k * NQKV + n0: k * NQKV + n0 + nw],
                            start=(k == 0), stop=(k == 2),
                        )
                    nc.scalar.copy(y_sb[:, n0:n0 + nw], ps[:, :nw])
                nc.sync.dma_start(yv[m * 128:(m + 1) * 128, :], y_sb[:])

            mrow_sb = wts.tile([128, 9], fp32, tag="mrow")
            nc.sync.dma_start(mrow_sb[:], mrow[:])
            id_sb = wts.tile([128, 128], fp32, tag="ident")
            nc.sync.dma_start(id_sb[:], ident[:])
            wp_sb = wts.tile([128, 3 * D], fp32, tag="wp")
            nc.sync.dma_start(wp_sb[:].rearrange("p (k c) -> p k c", c=D),
                              wp[:].rearrange("(k p) c -> p k c", p=128))

            outcat = [wts.tile([128, N], fp32, tag=f"outcat{g}", name=f"outcat{g}")
                      for g in range(3)]
            yflat = ydram[:]

            # ---------------- stage 2: per-head attention -------------------
            for h in range(NUM_HEADS):
                rows = []
                for s in range(3):
                    off = s * SLAB_ROWS * NQKV + 384 * s + h * CHUNK
                    r_sb = sm.tile([128, 256], fp32, tag=f"rows{s}")
                    nc.sync.dma_start(
                        r_sb[:].rearrange("p (t c) -> p t c", c=32),
                        yflat[off:off + CHUNK].rearrange("(t p c) -> p t c",
                                                         p=128, c=32))
                    rows.append(r_sb)

                # --- selected q rows + row norms (recip) ---
                qrow = sm.tile([128, 256], fp32, tag="qrow")
                nc.vector.tensor_scalar_mul(qrow[:], rows[0][:], mrow_sb[:, 0:1])
                nc.vector.scalar_tensor_tensor(qrow[:], rows[1][:], mrow_sb[:, 1:2],
                                               qrow[:], op0=AO.mult, op1=AO.add)
                nc.vector.scalar_tensor_tensor(qrow[:], rows[2][:], mrow_sb[:, 2:3],
                                               qrow[:], op0=AO.mult, op1=AO.add)
                sqq = sm.tile([128, 256], fp32, tag="sqq")
                nc.vector.tensor_mul(sqq[:], qrow[:], qrow[:])
                rq_all = sm.tile([128, 8], fp32, tag="rq")
                nc.vector.tensor_reduce(rq_all[:],
                                        sqq[:].rearrange("p (t c) -> p t c", c=32),
                                        axis=mybir.AxisListType.X, op=AO.add)
                nc.scalar.activation(rq_all[:], rq_all[:], AF.Sqrt)
                nc.vector.reciprocal(rq_all[:], rq_all[:])

                # --- selected k rows, normalized in row layout ---
                krow = sm.tile([128, 256], fp32, tag="krow")
                nc.vector.tensor_scalar_mul(krow[:], rows[0][:], mrow_sb[:, 3:4])
                nc.vector.scalar_tensor_tensor(krow[:], rows[1][:], mrow_sb[:, 4:5],
                                               krow[:], op0=AO.mult, op1=AO.add)
                nc.vector.scalar_tensor_tensor(krow[:], rows[2][:], mrow_sb[:, 5:6],
                                               krow[:], op0=AO.mult, op1=AO.add)
                sqk = sm.tile([128, 256], fp32, tag="sqk")
                nc.vector.tensor_mul(sqk[:], krow[:], krow[:])
                rk_all = sm.tile([128, 8], fp32, tag="rk")
                nc.vector.tensor_reduce(rk_all[:],
                                        sqk[:].rearrange("p (t c) -> p t c", c=32),
                                        axis=mybir.AxisListType.X, op=AO.add)
                nc.scalar.activation(rk_all[:], rk_all[:], AF.Sqrt)
                nc.vector.reciprocal(rk_all[:], rk_all[:])
                for t in range(8):
                    nc.vector.tensor_scalar_mul(krow[:, 32 * t:32 * (t + 1)],
                                                krow[:, 32 * t:32 * (t + 1)],
                                                rk_all[:, t:t + 1])

                # --- PE-transpose q,k row blocks -> qT [32,1024], knT [32,1040]
                qT = sm.tile([32, N], fp32, tag="qT")
                knT = sm.tile([32, M_FULL], fp32, tag="knT")
                for t in range(8):
                    pst = sps.tile([128, 512], fp32, tag="ps_small")
                    nc.tensor.transpose(pst[:32, :128], qrow[:, 32 * t:32 * (t + 1)],
                                        id_sb[:])
                    nc.scalar.copy(qT[:, 128 * t:128 * (t + 1)], pst[:32, :128])
                    psk = sps.tile([128, 512], fp32, tag="ps_small")
                    nc.tensor.transpose(psk[:32, :128], krow[:, 32 * t:32 * (t + 1)],
                                        id_sb[:])
                    nc.scalar.copy(knT[:, 128 * t:128 * (t + 1)], psk[:32, :128])
                nc.sync.dma_start(knT[:, N:], memknt[32 * h:32 * h + 32, :])

                # --- selected v rows (bf16) + mem_v ---
                v_bf = sm.tile([128, 9 * 32], bf16, tag="vbf")
                nc.vector.tensor_scalar_mul(v_bf[:, :256], rows[0][:],
                                            mrow_sb[:, 6:7])
                nc.vector.scalar_tensor_tensor(v_bf[:, :256], rows[1][:],
                                               mrow_sb[:, 7:8], v_bf[:, :256],
                                               op0=AO.mult, op1=AO.add)
                nc.vector.scalar_tensor_tensor(v_bf[:, :256], rows[2][:],
                                               mrow_sb[:, 8:9], v_bf[:, :256],
                                               op0=AO.mult, op1=AO.add)
                nc.vector.memset(v_bf[:, 256:], 0.0)
                nc.sync.dma_start(v_bf[:NUM_MEM, 256:288],
                                  memv[NUM_MEM * h:NUM_MEM * (h + 1), :])

                rs_all = sm.tile([128, 8], fp32, tag="rs")
                adram = dr2.tile([N, M_PAD], bf16, tag="adram")
                av = adram[:]

                # --- per row-tile: scores -> topk -> attn rows -> adram ---
                for rt in range(8):
                    ps_s = bps.tile([128, M_FULL], fp32, tag="ps_s")
                    lhs = qT[:, rt * 128:(rt + 1) * 128]
                    for (n0, nw) in ((0, 512), (512, 512), (1024, 16)):
                        nc.tensor.matmul(ps_s[:, n0:n0 + nw], lhs,
                                         knT[:, n0:n0 + nw], start=True, stop=True)
                    sc = scp.tile([128, M_FULL], fp32, tag="sc")
                    nc.scalar.copy(sc[:], ps_s[:])
                    nc.vector.scalar_tensor_tensor(
                        sc[:, rt * 128:(rt + 1) * 128], id_sb[:], NEG,
                        sc[:, rt * 128:(rt + 1) * 128],
                        op0=AO.mult, op1=AO.add)
                    m8a = sm.tile([128, 8], fp32, tag="m8a")
                    m8b = sm.tile([128, 8], fp32, tag="m8b")
                    m8h = sm.tile([128, 8], fp32, tag="m8h")
                    scw = scp.tile([128, M_FULL], fp32, tag="scw")
                    nc.vector.max(m8a[:], sc[:])
                    nc.vector.match_replace(scw[:], m8a[:], sc[:], NEG)
                    for r in range(6):
                        nc.vector.max(m8b[:], scw[:])
                        nc.vector.match_replace(scw[:], m8b[:], scw[:], NEG)
                    nc.vector.max(m8h[:], scw[:])

                    rq = rq_all[:, rt:rt + 1]
                    sc_ap = sm.tile([128, 1], fp32, tag="scl")
                    nc.vector.tensor_scalar_mul(sc_ap[:], rq, float(scale_vals[h]))
                    bias = sm.tile([128, 1], fp32, tag="bias")
                    nc.vector.scalar_tensor_tensor(bias[:], m8a[:, 0:1], -1.0,
                                                   sc_ap[:], op0=AO.mult,
                                                   op1=AO.mult)
                    ex = scp.tile([128, M_FULL], fp32, tag="ex")
                    nc.scalar.activation(ex[:], sc[:], AF.Exp,
                                         bias=bias[:], scale=sc_ap[:])
                    attn = attp.tile([128, M_PAD], bf16, tag="attn")
                    nc.vector.scalar_tensor_tensor(
                        attn[:, :M_FULL], sc[:], m8h[:, 7:8], ex[:],
                        op0=AO.is_ge, op1=AO.mult,
                        accum_out=rs_all[:, rt:rt + 1])
                    nc.vector.memset(attn[:, M_FULL:], 0.0)
                    rsr = sm.tile([128, 1], fp32, tag="rsr")
                    nc.vector.reciprocal(rsr[:], rs_all[:, rt:rt + 1])
                    nc.vector.tensor_scalar_mul(attn[:, :M_FULL], attn[:, :M_FULL],
                                                rsr[:])
                    nc.sync.dma_start(av[rt * 128:(rt + 1) * 128, :], attn[:])

                # --- attn.T via xbar transpose; attn@v on PE ---
                g, slot = h // 4, h % 4
                aT = []
                for mt in range(9):
                    a_sb = attp.tile([128, N], bf16, tag=f"aT{mt}", name=f"aT{mt}")
                    nc.scalar.dma_start_transpose(a_sb[:],
                                                  av[:, mt * 128:(mt + 1) * 128])
                    aT.append(a_sb)
                for half in range(2):
                    c0 = half * 512
                    ps_o = sps.tile([128, 512], fp32, tag="ps_small")
                    for mt in range(9):
                        nc.tensor.matmul(ps_o[:32, :],
                                         v_bf[:, mt * 32:(mt + 1) * 32],
                                         aT[mt][:, c0:c0 + 512],
                                         start=(mt == 0), stop=(mt == 8))
                    nc.scalar.copy(outcat[g][32 * slot:32 * slot + 32, c0:c0 + 512],
                                   ps_o[:32, :])

            # ---------------- stage 3: projection ---------------------------
            for et in range(3):
                f_sb = st1.tile([128, N], fp32, tag="f")
                for half in range(2):
                    c0 = half * 512
                    ps_f = sps.tile([128, 512], fp32, tag="ps_small")
                    for g in range(3):
                        nc.tensor.matmul(ps_f[:],
                                         wp_sb[:, g * D + et * 128:
                                               g * D + (et + 1) * 128],
                                         outcat[g][:, c0:c0 + 512],
                                         start=(g == 0), stop=(g == 2))
                    nc.scalar.copy(f_sb[:, c0:c0 + 512], ps_f[:])
                nc.sync.dma_start(out_ext[et * 128:(et + 1) * 128, :], f_sb[:])

    nc.compile()
    return nc


def kernel(x, w_qkv, w_proj, scale, mem_k, mem_v):
    from concourse.bass_utils import run_bass_kernel_spmd
    import ml_dtypes

    x = np.asarray(x, np.float32)
    w_qkv = np.asarray(w_qkv, np.float32)
    w_proj = np.asarray(w_proj, np.float32)
    scale = np.asarray(scale, np.float32)
    mem_k = np.asarray(mem_k, np.float32)
    mem_v = np.asarray(mem_v, np.float32)

    scale_vals = scale.reshape(-1)
    assert scale_vals.shape[0] == NUM_HEADS

    x_flat = x.reshape(B * N, D)
    wq_in = np.ascontiguousarray(w_qkv.T)
    wp_in = np.ascontiguousarray(w_proj.T)

    mkn = mem_k / np.maximum(
        np.linalg.norm(mem_k, axis=-1, keepdims=True), 1e-12)
    memknt = np.ascontiguousarray(
        mkn.transpose(0, 2, 1).reshape(NUM_HEADS * 32, NUM_MEM)).astype(np.float32)
    memv_in = mem_v.reshape(NUM_HEADS * NUM_MEM, 32).astype(ml_dtypes.bfloat16)
    ident = np.eye(128, dtype=np.float32)

    in_maps = []
    for j in range(8):
        xp = np.zeros((XPAD, D), np.float32)
        mrow = np.zeros((128, 9), np.float32)
        for g in range(3):  # 0=q 1=k 2=v
            s = (2 * g + j) % 3
            gstart = (g * 96 + j * 12) * CHUNK
            r0 = gstart // NQKV
            assert gstart - r0 * NQKV == 384 * s, (j, g, s)
            nrows = min(SLAB_ROWS, B * N - r0)
            xp[s * SLAB_ROWS: s * SLAB_ROWS + nrows] = x_flat[r0:r0 + nrows]
            if g == 0:
                mrow[:, s] = 1.0  # q role mask, cols 0..2
            elif g == 1:
                mrow[:, 3 + s] = 1.0  # k role mask, cols 3..5
            else:
                mrow[:, 6 + s] = 1.0  # v role mask, cols 6..8
        in_maps.append({
            "xt": np.ascontiguousarray(xp.T),
            "wq": wq_in, "wp": wp_in, "mrow": mrow, "ident": ident,
            "memknt": memknt, "memv": memv_in,
        })

    nc = _build_graph(scale_vals)
    import os
    trace = os.environ.get("KERNEL_TRACE", "1") == "1"
    try:
        res = run_bass_kernel_spmd(nc, in_maps, core_ids=list(range(8)),
                                   trace=trace)
    except Exception:
        res = run_bass_kernel_spmd(nc, in_maps, core_ids=list(range(8)))
    STATS["exec_time_ns"] = getattr(res, "exec_time_ns", None)

    outs = res.results
    full = np.zeros((B, Hh, Ww, D), np.float32)
    for j in range(8):
        o = outs[j]["out"] if isinstance(outs[j], dict) else outs[j]
        full[j] = np.asarray(o, np.float32).T.reshape(Hh, Ww, D)
    return full
